# revision 38
# baseline (speedup 1.0000x reference)
"""EGNN layer (fully-connected graph, N=384, H=256) on 8 TRN2 NeuronCores.

Sharding: receivers are split 48 per core. Each core computes, for its 48
receivers i and all 384 senders j (self-edge included, corrected exactly):

  h1(j,i)  = silu( eW1a@n_i + eW1b@n_j + w1c*radial(i,j) + b1 )
  msg(j,i) = silu( eW2@h1 + b2 )
  agg(i)   = sum_j msg(j,i) - msg(i,i)
  nodes'   = node MLP + residual (for the 48 local nodes)
  p1(j,i)  = silu( pW1@msg + pb1 );  psum4 += (pW2*[p_i|1]).T @ p1
  pos update: sum_i (p_j - p_i)*scale = p_j*colsum - P^T S (clip never binds;
  pb2's linear contribution is added exactly on the host)

The radial term is folded into the tensor engine:
  w1c_k * radial(i,j) = w1c_k q_i (receiver bias via scalar op) + w1c_k q_j
  (folded into the sender tables B') - 2 w1c_k <p_i, p_j> (K=3 matmul).

Hot-path matmuls (per-edge stages) run in bf16 (fp32-accumulated); the
one-time precompute (sender tables, receiver biases, node MLP, pos
products) runs in float32r. Only [256,48] nodes and [4,384] pos products
leave each core; the host does the final concat / 8-way reduction.
"""

import sys

sys.path.insert(0, "/opt/trn_rl_repo")

import numpy as np

import concourse.bass as bass
import concourse.mybir as mybir
import concourse.tile as tile
from concourse.bass import ds
from concourse.bass_utils import run_bass_kernel_spmd

F32 = mybir.dt.float32
F32R = mybir.dt.float32r
BF16 = mybir.dt.bfloat16
ACTF = mybir.ActivationFunctionType
ALU = mybir.AluOpType
AX = mybir.AxisListType

N = 384          # nodes
H = 256          # hidden
NCORES = 8
NB = N // NCORES  # receivers per core (48)
G = 6            # receivers per ACT batch group
NGRP = NB // G
P = 128          # partitions

# packA column offsets
_PA = {}
_off = 0
for _nm, _w in [("ndT", 768), ("ndrT", 96), ("w1aT", 512), ("w1bT", 512),
                ("w2T", 512), ("pw1T", 512), ("nw1T", 1024), ("nw2T", 512),
                ("eb1", 2), ("eb2", 2), ("pb1", 2), ("nb1", 2), ("nb2", 2)]:
    _PA[_nm] = _off
    _off += _w
CA = _off
_PB = {"posT": 0, "posrT": 384, "w1c": 432, "pw2row": 688, "pr4": 944}
CB = 1136


def _split_waits(nc, max_waits=1):
    """walrus in this container allows 1 inline sync-wait per instruction;
    move extra waits onto same-engine NoOp carriers inserted just before."""
    n = 0
    for f in nc.m.functions:
        for blk in f.blocks:
            out = []
            for inst in blk.instructions:
                si = getattr(inst, "sync_info", None)
                if si is not None and si.on_wait and len(si.on_wait) > max_waits:
                    waits = list(si.on_wait)
                    extra, keep = waits[:-max_waits], waits[-max_waits:]
                    for j, w in enumerate(extra):
                        d = mybir.InstNoOp(
                            name=f"{inst.name}-wsplit{j}", ins=[], outs=[])
                        d.engine = inst.engine
                        d.sync_info = mybir.SyncInfo(on_wait=[w], on_update=[])
                        out.append(d)
                    inst.sync_info = mybir.SyncInfo(
                        on_wait=keep, on_update=list(si.on_update or []))
                    n += 1
                out.append(inst)
            blk.instructions = out
    return n


def build():
    nc = bass.Bass()
    dp = nc.declare_dram_parameter

    # ---- inputs (per-core; host marshals layouts) ----
    # packA columns: ndT(768) ndrT(96) w1aT(512) w1bT(512) w2T(512)
    # pw1T(512) nw1T(1024) nw2T(512) eb1 eb2 pb1 nb1 nb2 (2 each)
    packA_e = dp("packA", [P, CA], F32, isOutput=False)
    # packB: rows 0-2: posT(384) posrT(48); row 0: w1c(256) pw2row(256) pr4(192)
    packB_e = dp("packB", [3, CB], F32, isOutput=False)

    # ---- outputs ----
    noutT_e = dp("nodes_outT", [H, NB], F32, isOutput=True)
    pout_e = dp("pos_out", [4, N], F32, isOutput=True)

    with tile.TileContext(nc) as tc:
        with tc.tile_pool(name="const", bufs=1) as cp, \
             tc.tile_pool(name="stage", bufs=2) as stp, \
             tc.tile_pool(name="l3p", bufs=8) as l3p, \
             tc.tile_pool(name="psum", bufs=2, space="PSUM") as pp:

            dma = nc.sync.dma_start

            # preload the Silu spline tables while DMAs run
            warmt = cp.tile([1, 1], F32, tag="warmt", name="warmt")
            nc.vector.memset(warmt, 0.0)
            nc.scalar.activation(warmt, warmt, ACTF.Silu)

            # ============ loads ============
            # packB first (tiny, gates the q-path / stage-1); packA split in
            # use-order chunks across DMA queues so transfers parallelize
            packB = cp.tile([3, CB], F32, tag="packB", name="packB")
            dma(out=packB, in_=packB_e[:, :])
            packA = cp.tile([P, CA], F32, tag="packA", name="packA")
            c1 = _PA["w2T"]            # ndT ndrT w1aT w1bT
            c2 = _PA["nw1T"]           # w2T pw1T
            for o, w in ((0, c1), (c1, c2 - c1), (c2, CA - c2)):
                dma(out=packA[:, ds(o, w)], in_=packA_e[:, ds(o, w)])

            def pa(nm, w, i=0):
                return packA[:, ds(_PA[nm] + i * w, w)]

            ndT_f = [pa("ndT", N, h) for h in range(2)]
            ndrT_f = [pa("ndrT", NB, h) for h in range(2)]
            w1aT_f = [pa("w1aT", H, kc) for kc in range(2)]
            w1bT_f = [pa("w1bT", H, kc) for kc in range(2)]
            w2T_f = [pa("w2T", H, kc) for kc in range(2)]
            pw1T_f = [pa("pw1T", H, kc) for kc in range(2)]
            nw1T_f = [pa("nw1T", H, kc) for kc in range(4)]
            nw2T_f = [pa("nw2T", H, kc) for kc in range(2)]
            eb1c = [pa("eb1", 1, h) for h in range(2)]
            eb2c = [pa("eb2", 1, h) for h in range(2)]
            pb1c = [pa("pb1", 1, h) for h in range(2)]
            nb1c = [pa("nb1", 1, h) for h in range(2)]
            nb2c = [pa("nb2", 1, h) for h in range(2)]
            posT_f = packB[:, ds(_PB["posT"], N)]
            posrT_f = packB[:, ds(_PB["posrT"], NB)]
            w1c_f = packB[0:1, ds(_PB["w1c"], H)]
            pw2r_f = packB[0:1, ds(_PB["pw2row"], H)]
            pr4_f = packB[0:1, ds(_PB["pr4"], 4 * NB)]

            # ===== typed working copies (f32r for precompute, bf16 hot) =====
            def to_t(src_tiles, name, dt):
                out = []
                for i, s in enumerate(src_tiles):
                    t = cp.tile(list(s.shape), dt, tag=f"{name}{i}c",
                                name=f"{name}{i}c")
                    nc.vector.tensor_copy(t, s)
                    out.append(t)
                return out

            # q-path + stage-1-critical casts first (packB lands early);
            # bulk weight casts queue after
            posT_b = to_t([posT_f], "posTb", BF16)[0]
            w1c_r = to_t([w1c_f], "w1c", F32R)[0]

            ones31_f = cp.tile([3, 1], F32, tag="ones31", name="ones31")
            nc.vector.memset(ones31_f, 1.0)
            ones31_r = to_t([ones31_f], "ones31", F32R)[0]
            ones13_f = cp.tile([1, 3], F32, tag="ones13", name="ones13")
            nc.vector.memset(ones13_f, 1.0)
            ones13_r = to_t([ones13_f], "ones13", F32R)[0]

            sq_r = cp.tile([3, N], F32R, tag="sq", name="sq")
            nc.vector.tensor_mul(sq_r, posT_f, posT_f)
            q_ps = pp.tile([1, N], F32, tag="s1", name="sclq", bufs=3)
            nc.tensor.matmul(q_ps, ones31_r, sq_r, start=True, stop=True)
            q_r = cp.tile([1, N], F32R, tag="qrow", name="qrow")
            nc.vector.tensor_copy(q_r, q_ps)

            sqloc_r = cp.tile([3, NB], F32R, tag="sqloc", name="sqloc")
            nc.vector.tensor_mul(sqloc_r, posrT_f, posrT_f)
            qloc_ps = pp.tile([1, NB], F32, tag="s1", name="sclq", bufs=3)
            nc.tensor.matmul(qloc_ps, ones31_r, sqloc_r, start=True, stop=True)
            qloc_r = cp.tile([1, NB], F32R, tag="qloc", name="qloc")
            nc.vector.tensor_copy(qloc_r, qloc_ps)

            # w1c broadcast to 3 partitions (for per-receiver cross lhsT)
            w1cb3_ps = pp.tile([3, H], F32, tag="s1", name="sclq", bufs=3)
            nc.tensor.matmul(w1cb3_ps, ones13_r, w1c_r, start=True, stop=True)
            w1cb3_b = cp.tile([3, H], BF16, tag="w1cb3", name="w1cb3")
            nc.vector.tensor_copy(w1cb3_b, w1cb3_ps)

            ndT_r = to_t(ndT_f, "ndT", F32R)
            ndrT_r = to_t(ndrT_f, "ndrT", F32R)
            w1aT_r = to_t(w1aT_f, "w1aT", F32R)
            w1bT_r = to_t(w1bT_f, "w1bT", F32R)
            w2T_b = to_t(w2T_f, "w2T", BF16)
            pw1T_b = to_t(pw1T_f, "pw1T", BF16)
            nw1T_r = to_t(nw1T_f, "nw1T", F32R)
            nw2T_r = to_t(nw2T_f, "nw2T", F32R)
            pw2r_r = to_t([pw2r_f], "pw2row", F32R)[0]
            pr4_r = to_t([pr4_f], "pr4", F32R)[0]
            w1abT_r = []
            for kc in range(2):
                t = cp.tile([P, H], F32R, tag=f"w1abT{kc}r",
                            name=f"w1abT{kc}r")
                nc.vector.tensor_add(t, w1aT_f[kc], w1bT_f[kc])
                w1abT_r.append(t)

            # ============ sender tables B'[k,j] = eW1b@n_j + w1c_k q_j ======
            BTp_f = []
            for h in range(2):
                ps = pp.tile([P, N], F32, tag="s1", name="s1", bufs=3)
                nc.tensor.matmul(ps, w1bT_r[0][:, ds(P * h, P)], ndT_r[0],
                                 start=True, stop=False)
                nc.tensor.matmul(ps, w1bT_r[1][:, ds(P * h, P)], ndT_r[1],
                                 start=False, stop=False)
                nc.tensor.matmul(ps, w1c_r[:, ds(P * h, P)], q_r,
                                 start=False, stop=True)
                t = cp.tile([P, N], F32, tag=f"BTp{h}", name=f"BTp{h}")
                nc.vector.tensor_copy(t, ps)
                BTp_f.append(t)

            # ==== receiver bias A'[k,i] = eW1a@n_i + b1 + w1c_k q_i (local) ====
            A2loc_f = []
            for h in range(2):
                ps = pp.tile([P, NB], F32, tag="s1", name="s1", bufs=3)
                nc.tensor.matmul(ps, w1aT_r[0][:, ds(P * h, P)], ndrT_r[0],
                                 start=True, stop=False)
                nc.tensor.matmul(ps, w1aT_r[1][:, ds(P * h, P)], ndrT_r[1],
                                 start=False, stop=False)
                nc.tensor.matmul(ps, w1c_r[:, ds(P * h, P)], qloc_r,
                                 start=False, stop=True)
                t = cp.tile([P, NB], F32, tag=f"A2loc{h}", name=f"A2loc{h}")
                nc.vector.tensor_scalar_add(t, ps, eb1c[h])
                A2loc_f.append(t)

            # ============ self messages msg(i,i) (radial = 0 exactly) ======
            h1self_b = []
            for h in range(2):
                ps = pp.tile([P, NB], F32, tag="s1", name="s1", bufs=3)
                nc.tensor.matmul(ps, w1abT_r[0][:, ds(P * h, P)], ndrT_r[0],
                                 start=True, stop=False)
                nc.tensor.matmul(ps, w1abT_r[1][:, ds(P * h, P)], ndrT_r[1],
                                 start=False, stop=True)
                t = cp.tile([P, NB], BF16, tag=f"h1self{h}", name=f"h1self{h}")
                nc.scalar.activation(t, ps, ACTF.Silu, bias=eb1c[h])
                h1self_b.append(t)
            msgself_f = []
            for h in range(2):
                ps = pp.tile([P, NB], F32, tag="p1", name="p1")
                nc.tensor.matmul(ps, w2T_b[0][:, ds(P * h, P)], h1self_b[0],
                                 start=True, stop=False)
                nc.tensor.matmul(ps, w2T_b[1][:, ds(P * h, P)], h1self_b[1],
                                 start=False, stop=True)
                t = cp.tile([P, NB], F32, tag=f"msgself{h}", name=f"msgself{h}")
                nc.scalar.activation(t, ps, ACTF.Silu, bias=eb2c[h])
                msgself_f.append(t)

            # ==== pos-update lhsT: M4[kc][k, 4i:4i+4] = pW2[k]*[p_i|1] ====
            M4_b = []
            for kc in range(2):
                ps = pp.tile([P, 4 * NB], F32, tag="s1", name="m4ps", bufs=3)
                nc.tensor.matmul(ps, pw2r_r[:, ds(P * kc, P)], pr4_r,
                                 start=True, stop=True)
                t = cp.tile([P, 4 * NB], BF16, tag=f"M4_{kc}", name=f"M4_{kc}")
                nc.vector.tensor_copy(t, ps)
                M4_b.append(t)
            p4ps = pp.tile([4, N], F32, tag="p4", name="p4ps", bufs=1)

            # ============ edge pipeline ============
            aggT_f = [cp.tile([P, NB], F32, tag=f"aggT{h}", name=f"aggT{h}")
                      for h in range(2)]

            for g in range(NGRP):
                h1stg = [stp.tile([P, G * N], BF16, tag=f"h1stg{h}",
                                  name=f"h1stg{h}") for h in range(2)]
                msgstg = [stp.tile([P, G * N], BF16, tag=f"msgstg{h}",
                                   name=f"msgstg{h}") for h in range(2)]
                p1stg = [stp.tile([P, G * N], BF16, tag=f"p1stg{h}",
                                  name=f"p1stg{h}") for h in range(2)]

                # stage 1: h1_pre = cross(K=3 matmul) + A'col + B'
                for i in range(G):
                    iloc = G * g + i
                    l3 = l3p.tile([3, H], BF16, tag="lhsT3", name="lhsT3")
                    nc.gpsimd.tensor_scalar(
                        out=l3, in0=w1cb3_b,
                        scalar1=posrT_f[:, ds(iloc, 1)], scalar2=-2.0,
                        op0=ALU.mult, op1=ALU.mult)
                    for h in range(2):
                        ps = pp.tile([P, N], F32, tag="s1", name="s1", bufs=3)
                        nc.tensor.matmul(ps, l3[:, ds(P * h, P)], posT_b,
                                         start=True, stop=True)
                        nc.vector.scalar_tensor_tensor(
                            out=h1stg[h][:, ds(i * N, N)], in0=ps,
                            scalar=A2loc_f[h][:, ds(iloc, 1)], in1=BTp_f[h],
                            op0=ALU.add, op1=ALU.add)
                for h in range(2):
                    nc.scalar.activation(h1stg[h], h1stg[h], ACTF.Silu)

                # stage 2: msg = silu(eW2@h1 + b2) straight from PSUM on ACT;
                # accum_out computes agg = sum_j msg for free
                for i in range(G):
                    iloc = G * g + i
                    for h in range(2):
                        ps = pp.tile([P, N], F32, tag="mm", name="mm")
                        nc.tensor.matmul(ps, w2T_b[0][:, ds(P * h, P)],
                                         h1stg[0][:, ds(i * N, N)],
                                         start=True, stop=False)
                        nc.tensor.matmul(ps, w2T_b[1][:, ds(P * h, P)],
                                         h1stg[1][:, ds(i * N, N)],
                                         start=False, stop=True)
                        if i < 2:
                            # shed the accumulator read to DVE's slack
                            nc.scalar.activation(
                                msgstg[h][:, ds(i * N, N)], ps, ACTF.Silu,
                                bias=eb2c[h])
                            nc.vector.tensor_reduce(
                                aggT_f[h][:, ds(iloc, 1)],
                                msgstg[h][:, ds(i * N, N)], AX.X, ALU.add)
                        else:
                            nc.scalar.activation(
                                msgstg[h][:, ds(i * N, N)], ps, ACTF.Silu,
                                bias=eb2c[h],
                                accum_out=aggT_f[h][:, ds(iloc, 1)])

                # stage 3: p1_pre = pW1 @ msg over 512-wide windows
                # (receiver-agnostic; fewer matmuls, amortized weight loads)
                W3 = 512
                for w0 in range(0, G * N, W3):
                    wl = min(W3, G * N - w0)
                    for h in range(2):
                        ps = pp.tile([P, W3], F32, tag="p1", name="p1")
                        nc.tensor.matmul(ps[:, 0:wl],
                                         pw1T_b[0][:, ds(P * h, P)],
                                         msgstg[0][:, ds(w0, wl)],
                                         start=True, stop=False)
                        nc.tensor.matmul(ps[:, 0:wl],
                                         pw1T_b[1][:, ds(P * h, P)],
                                         msgstg[1][:, ds(w0, wl)],
                                         start=False, stop=True)
                        nc.vector.tensor_copy(p1stg[h][:, ds(w0, wl)],
                                              ps[:, 0:wl])
                if g == NGRP - 1:
                    hn = G * N // 2
                    for h in range(2):
                        nc.scalar.activation(p1stg[h][:, 0:hn],
                                             p1stg[h][:, 0:hn],
                                             ACTF.Silu, bias=pb1c[h])
                    for h in range(2):
                        nc.scalar.activation(p1stg[h][:, ds(hn, hn)],
                                             p1stg[h][:, ds(hn, hn)],
                                             ACTF.Silu, bias=pb1c[h])
                else:
                    for h in range(2):
                        nc.scalar.activation(p1stg[h], p1stg[h], ACTF.Silu,
                                             bias=pb1c[h])

                # stage 4: psum4 += M4_i.T @ p1_i  (pb2 handled on host)
                for i in range(G):
                    iloc = G * g + i
                    first = (iloc == 0)
                    last = (iloc == NB - 1)
                    nc.tensor.matmul(p4ps, M4_b[0][:, ds(4 * iloc, 4)],
                                     p1stg[0][:, ds(i * N, N)],
                                     start=first, stop=False,
                                     skip_group_check=True)
                    nc.tensor.matmul(p4ps, M4_b[1][:, ds(4 * iloc, 4)],
                                     p1stg[1][:, ds(i * N, N)],
                                     start=False, stop=last,
                                     skip_group_check=True)

            # ============ node MLP + residual ============
            # agg := agg - msg_self, fused with the f32r cast
            aggT_r = []
            for h in range(2):
                t = cp.tile([P, NB], F32R, tag=f"aggT{h}c", name=f"aggT{h}c")
                nc.vector.tensor_sub(t, aggT_f[h], msgself_f[h])
                aggT_r.append(t)
            nh_r = []
            for h in range(2):
                ps = pp.tile([P, NB], F32, tag="s1", name="s1", bufs=3)
                nc.tensor.matmul(ps, nw1T_r[0][:, ds(P * h, P)], ndrT_r[0],
                                 start=True, stop=False)
                nc.tensor.matmul(ps, nw1T_r[1][:, ds(P * h, P)], ndrT_r[1],
                                 start=False, stop=False)
                nc.tensor.matmul(ps, nw1T_r[2][:, ds(P * h, P)], aggT_r[0],
                                 start=False, stop=False)
                nc.tensor.matmul(ps, nw1T_r[3][:, ds(P * h, P)], aggT_r[1],
                                 start=False, stop=True)
                t = cp.tile([P, NB], F32R, tag=f"nh{h}", name=f"nh{h}")
                nc.scalar.activation(t, ps, ACTF.Silu, bias=nb1c[h])
                nh_r.append(t)
            for h in range(2):
                ps = pp.tile([P, NB], F32, tag="p1", name="p1")
                nc.tensor.matmul(ps, nw2T_r[0][:, ds(P * h, P)], nh_r[0],
                                 start=True, stop=False)
                nc.tensor.matmul(ps, nw2T_r[1][:, ds(P * h, P)], nh_r[1],
                                 start=False, stop=True)
                t = cp.tile([P, NB], F32, tag=f"noutT{h}", name=f"noutT{h}")
                nc.vector.scalar_tensor_tensor(
                    out=t, in0=ps, scalar=nb2c[h], in1=ndrT_f[h],
                    op0=ALU.add, op1=ALU.add)
                dma(out=noutT_e[ds(P * h, P), :], in_=t)

            # ============ pos products out ============
            pout_f = cp.tile([4, N], F32, tag="pout", name="pout")
            nc.vector.tensor_copy(pout_f, p4ps)
            dma(out=pout_e[:, :], in_=pout_f)

    _split_waits(nc)
    return nc


_NC_CACHE = None


def _get_nc():
    global _NC_CACHE
    if _NC_CACHE is None:
        _NC_CACHE = build()
    return _NC_CACHE


def _make_in_maps(inputs):
    f32 = np.float32
    nodes = np.ascontiguousarray(np.asarray(inputs["nodes"], f32))
    pos = np.ascontiguousarray(np.asarray(inputs["pos"], f32))
    eW1 = np.asarray(inputs["eW1"], f32)
    eW2 = np.asarray(inputs["eW2"], f32)
    pW1 = np.asarray(inputs["pW1"], f32)
    pW2 = np.asarray(inputs["pW2"], f32)
    nW1 = np.asarray(inputs["nW1"], f32)
    nW2 = np.asarray(inputs["nW2"], f32)

    def c(x):
        return np.ascontiguousarray(x.astype(f32))

    def col2(v):
        return np.asarray(v, f32).reshape(H, 1).reshape(2, P).T  # [128, 2]

    eb1 = col2(inputs["eb1"]); eb2 = col2(inputs["eb2"])
    pb1 = col2(inputs["pb1"])
    nb1 = col2(inputs["nb1"]); nb2 = col2(inputs["nb2"])

    in_maps = []
    for cix in range(NCORES):
        blk = slice(NB * cix, NB * (cix + 1))
        packA = np.empty((P, CA), f32)

        def put(nm, arr):
            a = np.asarray(arr, f32)
            packA[:, _PA[nm]:_PA[nm] + a.shape[1]] = a

        ndT = nodes.T  # [256, 384]
        put("ndT", np.concatenate([ndT[:P], ndT[P:]], axis=1))
        ndrT = nodes[blk].T
        put("ndrT", np.concatenate([ndrT[:P], ndrT[P:]], axis=1))

        def wsplit(wT, nkc):  # [K, 256] -> [128, nkc*256]
            return np.concatenate([wT[P * k:P * (k + 1)] for k in range(nkc)],
                                  axis=1)

        put("w1aT", wsplit(eW1[:, :H].T, 2))
        put("w1bT", wsplit(eW1[:, H:2 * H].T, 2))
        put("w2T", wsplit(eW2.T, 2))
        put("pw1T", wsplit(pW1.T, 2))
        put("nw1T", wsplit(nW1.T, 4))
        put("nw2T", wsplit(nW2.T, 2))
        put("eb1", eb1); put("eb2", eb2); put("pb1", pb1)
        put("nb1", nb1); put("nb2", nb2)

        packB = np.zeros((3, CB), f32)
        packB[:, 0:N] = pos.T
        packB[:, _PB["posrT"]:_PB["posrT"] + NB] = pos[blk].T
        packB[0, _PB["w1c"]:_PB["w1c"] + H] = eW1[:, 2 * H]
        packB[0, _PB["pw2row"]:_PB["pw2row"] + H] = pW2.reshape(H)
        packB[0, _PB["pr4"]:_PB["pr4"] + 4 * NB] = np.concatenate(
            [pos[blk], np.ones((NB, 1), f32)], axis=1).reshape(4 * NB)
        in_maps.append({"packA": c(packA), "packB": c(packB)})
    return in_maps


def kernel(**inputs):
    f32 = np.float32
    nodes = np.ascontiguousarray(np.asarray(inputs["nodes"], f32))
    pos = np.ascontiguousarray(np.asarray(inputs["pos"], f32))
    in_maps = _make_in_maps(inputs)

    res = run_bass_kernel_spmd(_get_nc(), in_maps, list(range(NCORES))).results

    new_nodes = np.concatenate(
        [res[cix]["nodes_outT"].T for cix in range(NCORES)], axis=0)

    upd = np.zeros((N, 3), np.float64)
    pos64 = pos.astype(np.float64)
    for cix in range(NCORES):
        p4 = res[cix]["pos_out"].astype(np.float64)
        upd += pos64 * p4[3][:, None] - p4[0:3].T
    # pb2 enters every scale entry; its pos contribution is linear and exact:
    # sum_{i != j} (p_j - p_i) * pb2 = pb2 * (N * p_j - sum_i p_i)
    pb2 = float(np.asarray(inputs["pb2"]).reshape(-1)[0])
    upd += pb2 * (N * pos64 - pos64.sum(axis=0, keepdims=True))
    new_pos = (pos64 + upd).astype(f32)
    return new_nodes, new_pos


# revision 39
# speedup vs baseline: 1.0518x; 1.0518x over previous
"""EGNN layer (fully-connected graph, N=384, H=256) on 8 TRN2 NeuronCores.

Sharding: receivers are split 48 per core. Each core computes, for its 48
receivers i and all 384 senders j (self-edge included, corrected exactly):

  h1(j,i)  = silu( eW1a@n_i + eW1b@n_j + w1c*radial(i,j) + b1 )
  msg(j,i) = silu( eW2@h1 + b2 )
  agg(i)   = sum_j msg(j,i) - msg(i,i)
  nodes'   = node MLP + residual (for the 48 local nodes)
  p1(j,i)  = silu( pW1@msg + pb1 );  psum4 += (pW2*[p_i|1]).T @ p1
  pos update: sum_i (p_j - p_i)*scale = p_j*colsum - P^T S (clip never binds;
  pb2's linear contribution is added exactly on the host)

The radial term is folded into the tensor engine:
  w1c_k * radial(i,j) = w1c_k q_i (receiver bias via scalar op) + w1c_k q_j
  (folded into the sender tables B') - 2 w1c_k <p_i, p_j> (K=3 matmul).

Hot-path matmuls (per-edge stages) run in bf16 (fp32-accumulated); the
one-time precompute (sender tables, receiver biases, node MLP, pos
products) runs in float32r. Only [256,48] nodes and [4,384] pos products
leave each core; the host does the final concat / 8-way reduction.
"""

import sys

sys.path.insert(0, "/opt/trn_rl_repo")

import numpy as np

import concourse.bass as bass
import concourse.mybir as mybir
import concourse.tile as tile
from concourse.bass import ds
from concourse.bass_utils import run_bass_kernel_spmd

F32 = mybir.dt.float32
F32R = mybir.dt.float32r
BF16 = mybir.dt.bfloat16
ACTF = mybir.ActivationFunctionType
ALU = mybir.AluOpType
AX = mybir.AxisListType

N = 384          # nodes
H = 256          # hidden
NCORES = 8
NB = N // NCORES  # receivers per core (48)
G = 6            # receivers per ACT batch group
NGRP = NB // G
P = 128          # partitions

# packA column offsets
_PA = {}
_off = 0
for _nm, _w in [("ndT", 768), ("ndrT", 96), ("w1aT", 512), ("w1bT", 512),
                ("w2T", 512), ("pw1T", 512), ("nw1T", 1024), ("nw2T", 512),
                ("eb1", 2), ("eb2", 2), ("pb1", 2), ("nb1", 2), ("nb2", 2)]:
    _PA[_nm] = _off
    _off += _w
CA = _off
_PB = {"posT": 0, "posrT": 384, "w1c": 432, "pw2row": 688, "pr4": 944}
CB = 1136


def _split_waits(nc, max_waits=1):
    """walrus in this container allows 1 inline sync-wait per instruction;
    move extra waits onto same-engine NoOp carriers inserted just before."""
    n = 0
    for f in nc.m.functions:
        for blk in f.blocks:
            out = []
            for inst in blk.instructions:
                si = getattr(inst, "sync_info", None)
                if si is not None and si.on_wait and len(si.on_wait) > max_waits:
                    waits = list(si.on_wait)
                    extra, keep = waits[:-max_waits], waits[-max_waits:]
                    for j, w in enumerate(extra):
                        d = mybir.InstNoOp(
                            name=f"{inst.name}-wsplit{j}", ins=[], outs=[])
                        d.engine = inst.engine
                        d.sync_info = mybir.SyncInfo(on_wait=[w], on_update=[])
                        out.append(d)
                    inst.sync_info = mybir.SyncInfo(
                        on_wait=keep, on_update=list(si.on_update or []))
                    n += 1
                out.append(inst)
            blk.instructions = out
    return n


def build():
    nc = bass.Bass()
    dp = nc.declare_dram_parameter

    # ---- inputs (per-core; host marshals layouts) ----
    # packA columns: ndT(768) ndrT(96) w1aT(512) w1bT(512) w2T(512)
    # pw1T(512) nw1T(1024) nw2T(512) eb1 eb2 pb1 nb1 nb2 (2 each)
    packA_e = dp("packA", [P, CA], F32, isOutput=False)
    # packB: rows 0-2: posT(384) posrT(48); row 0: w1c(256) pw2row(256) pr4(192)
    packB_e = dp("packB", [3, CB], F32, isOutput=False)

    # ---- outputs ----
    noutT_e = dp("nodes_outT", [H, NB], F32, isOutput=True)
    pout_e = dp("pos_out", [4, N], F32, isOutput=True)

    with tile.TileContext(nc) as tc:
        with tc.tile_pool(name="const", bufs=1) as cp, \
             tc.tile_pool(name="stage", bufs=2) as stp, \
             tc.tile_pool(name="l3p", bufs=8) as l3p, \
             tc.tile_pool(name="psum", bufs=2, space="PSUM") as pp:

            dma = nc.sync.dma_start

            # preload the Silu spline tables while DMAs run
            warmt = cp.tile([1, 1], F32, tag="warmt", name="warmt")
            nc.vector.memset(warmt, 0.0)
            nc.scalar.activation(warmt, warmt, ACTF.Silu)

            # ============ loads ============
            # packB first (tiny, gates the q-path / stage-1); packA split in
            # use-order chunks across DMA queues so transfers parallelize
            packB = cp.tile([3, CB], F32, tag="packB", name="packB")
            dma(out=packB, in_=packB_e[:, :])
            packA = cp.tile([P, CA], F32, tag="packA", name="packA")
            c1 = _PA["w2T"]            # ndT ndrT w1aT w1bT
            c2 = _PA["nw1T"]           # w2T pw1T
            for o, w in ((0, c1), (c1, c2 - c1), (c2, CA - c2)):
                dma(out=packA[:, ds(o, w)], in_=packA_e[:, ds(o, w)])

            def pa(nm, w, i=0):
                return packA[:, ds(_PA[nm] + i * w, w)]

            ndT_f = [pa("ndT", N, h) for h in range(2)]
            ndrT_f = [pa("ndrT", NB, h) for h in range(2)]
            w1aT_f = [pa("w1aT", H, kc) for kc in range(2)]
            w1bT_f = [pa("w1bT", H, kc) for kc in range(2)]
            w2T_f = [pa("w2T", H, kc) for kc in range(2)]
            pw1T_f = [pa("pw1T", H, kc) for kc in range(2)]
            nw1T_f = [pa("nw1T", H, kc) for kc in range(4)]
            nw2T_f = [pa("nw2T", H, kc) for kc in range(2)]
            eb1c = [pa("eb1", 1, h) for h in range(2)]
            eb2c = [pa("eb2", 1, h) for h in range(2)]
            pb1c = [pa("pb1", 1, h) for h in range(2)]
            nb1c = [pa("nb1", 1, h) for h in range(2)]
            nb2c = [pa("nb2", 1, h) for h in range(2)]
            posT_f = packB[:, ds(_PB["posT"], N)]
            posrT_f = packB[:, ds(_PB["posrT"], NB)]
            w1c_f = packB[0:1, ds(_PB["w1c"], H)]
            pw2r_f = packB[0:1, ds(_PB["pw2row"], H)]
            pr4_f = packB[0:1, ds(_PB["pr4"], 4 * NB)]

            # ===== typed working copies (f32r for precompute, bf16 hot) =====
            def to_t(src_tiles, name, dt):
                out = []
                for i, s in enumerate(src_tiles):
                    t = cp.tile(list(s.shape), dt, tag=f"{name}{i}c",
                                name=f"{name}{i}c")
                    nc.vector.tensor_copy(t, s)
                    out.append(t)
                return out

            # q-path + stage-1-critical casts first (packB lands early);
            # bulk weight casts queue after
            posT_b = to_t([posT_f], "posTb", BF16)[0]
            w1c_r = to_t([w1c_f], "w1c", F32R)[0]

            ones31_f = cp.tile([3, 1], F32, tag="ones31", name="ones31")
            nc.vector.memset(ones31_f, 1.0)
            ones31_r = to_t([ones31_f], "ones31", F32R)[0]
            ones13_f = cp.tile([1, 3], F32, tag="ones13", name="ones13")
            nc.vector.memset(ones13_f, 1.0)
            ones13_r = to_t([ones13_f], "ones13", F32R)[0]

            sq_r = cp.tile([3, N], F32R, tag="sq", name="sq")
            nc.vector.tensor_mul(sq_r, posT_f, posT_f)
            q_ps = pp.tile([1, N], F32, tag="s1", name="sclq", bufs=3)
            nc.tensor.matmul(q_ps, ones31_r, sq_r, start=True, stop=True)
            q_r = cp.tile([1, N], F32R, tag="qrow", name="qrow")
            nc.vector.tensor_copy(q_r, q_ps)

            sqloc_r = cp.tile([3, NB], F32R, tag="sqloc", name="sqloc")
            nc.vector.tensor_mul(sqloc_r, posrT_f, posrT_f)
            qloc_ps = pp.tile([1, NB], F32, tag="s1", name="sclq", bufs=3)
            nc.tensor.matmul(qloc_ps, ones31_r, sqloc_r, start=True, stop=True)
            qloc_r = cp.tile([1, NB], F32R, tag="qloc", name="qloc")
            nc.vector.tensor_copy(qloc_r, qloc_ps)

            # w1c broadcast to 3 partitions (for per-receiver cross lhsT)
            w1cb3_ps = pp.tile([3, H], F32, tag="s1", name="sclq", bufs=3)
            nc.tensor.matmul(w1cb3_ps, ones13_r, w1c_r, start=True, stop=True)
            w1cb3_b = cp.tile([3, H], BF16, tag="w1cb3", name="w1cb3")
            nc.vector.tensor_copy(w1cb3_b, w1cb3_ps)

            ndT_r = to_t(ndT_f, "ndT", F32R)
            ndrT_r = to_t(ndrT_f, "ndrT", F32R)
            w1aT_r = to_t(w1aT_f, "w1aT", F32R)
            w1bT_r = to_t(w1bT_f, "w1bT", F32R)
            w2T_b = to_t(w2T_f, "w2T", BF16)
            pw1T_b = to_t(pw1T_f, "pw1T", BF16)
            pw2r_r = to_t([pw2r_f], "pw2row", F32R)[0]
            pr4_r = to_t([pr4_f], "pr4", F32R)[0]
            w1abT_r = []
            for kc in range(2):
                t = cp.tile([P, H], F32R, tag=f"w1abT{kc}r",
                            name=f"w1abT{kc}r")
                nc.vector.tensor_add(t, w1aT_f[kc], w1bT_f[kc])
                w1abT_r.append(t)

            # ============ sender tables B'[k,j] = eW1b@n_j + w1c_k q_j ======
            BTp_f = []
            for h in range(2):
                ps = pp.tile([P, N], F32, tag="s1", name="s1", bufs=3)
                nc.tensor.matmul(ps, w1bT_r[0][:, ds(P * h, P)], ndT_r[0],
                                 start=True, stop=False)
                nc.tensor.matmul(ps, w1bT_r[1][:, ds(P * h, P)], ndT_r[1],
                                 start=False, stop=False)
                nc.tensor.matmul(ps, w1c_r[:, ds(P * h, P)], q_r,
                                 start=False, stop=True)
                t = cp.tile([P, N], F32, tag=f"BTp{h}", name=f"BTp{h}")
                nc.vector.tensor_copy(t, ps)
                BTp_f.append(t)

            # ==== receiver bias A'[k,i] = eW1a@n_i + b1 + w1c_k q_i (local) ====
            A2loc_f = []
            for h in range(2):
                ps = pp.tile([P, NB], F32, tag="s1", name="s1", bufs=3)
                nc.tensor.matmul(ps, w1aT_r[0][:, ds(P * h, P)], ndrT_r[0],
                                 start=True, stop=False)
                nc.tensor.matmul(ps, w1aT_r[1][:, ds(P * h, P)], ndrT_r[1],
                                 start=False, stop=False)
                nc.tensor.matmul(ps, w1c_r[:, ds(P * h, P)], qloc_r,
                                 start=False, stop=True)
                t = cp.tile([P, NB], F32, tag=f"A2loc{h}", name=f"A2loc{h}")
                nc.vector.tensor_scalar_add(t, ps, eb1c[h])
                A2loc_f.append(t)

            # ============ self messages msg(i,i) (radial = 0 exactly) ======
            h1self_b = []
            for h in range(2):
                ps = pp.tile([P, NB], F32, tag="s1", name="s1", bufs=3)
                nc.tensor.matmul(ps, w1abT_r[0][:, ds(P * h, P)], ndrT_r[0],
                                 start=True, stop=False)
                nc.tensor.matmul(ps, w1abT_r[1][:, ds(P * h, P)], ndrT_r[1],
                                 start=False, stop=True)
                t = cp.tile([P, NB], BF16, tag=f"h1self{h}", name=f"h1self{h}")
                nc.scalar.activation(t, ps, ACTF.Silu, bias=eb1c[h])
                h1self_b.append(t)
            msgself_f = []
            for h in range(2):
                ps = pp.tile([P, NB], F32, tag="p1", name="p1")
                nc.tensor.matmul(ps, w2T_b[0][:, ds(P * h, P)], h1self_b[0],
                                 start=True, stop=False)
                nc.tensor.matmul(ps, w2T_b[1][:, ds(P * h, P)], h1self_b[1],
                                 start=False, stop=True)
                t = cp.tile([P, NB], F32, tag=f"msgself{h}", name=f"msgself{h}")
                nc.scalar.activation(t, ps, ACTF.Silu, bias=eb2c[h])
                msgself_f.append(t)

            # ==== pos-update lhsT: M4[kc][k, 4i:4i+4] = pW2[k]*[p_i|1] ====
            M4_b = []
            for kc in range(2):
                ps = pp.tile([P, 4 * NB], F32, tag="s1", name="m4ps", bufs=3)
                nc.tensor.matmul(ps, pw2r_r[:, ds(P * kc, P)], pr4_r,
                                 start=True, stop=True)
                t = cp.tile([P, 4 * NB], BF16, tag=f"M4_{kc}", name=f"M4_{kc}")
                nc.vector.tensor_copy(t, ps)
                M4_b.append(t)
            p4ps = pp.tile([4, N], F32, tag="p4", name="p4ps", bufs=1)

            # ============ edge pipeline ============
            aggT_f = [cp.tile([P, NB], F32, tag=f"aggT{h}", name=f"aggT{h}")
                      for h in range(2)]

            for g in range(NGRP):
                h1stg = [stp.tile([P, G * N], BF16, tag=f"h1stg{h}",
                                  name=f"h1stg{h}") for h in range(2)]
                msgstg = [stp.tile([P, G * N], BF16, tag=f"msgstg{h}",
                                   name=f"msgstg{h}") for h in range(2)]
                p1stg = [stp.tile([P, G * N], BF16, tag=f"p1stg{h}",
                                  name=f"p1stg{h}") for h in range(2)]

                # stage 1: h1_pre = cross(K=3 matmul) + A'col + B'
                for i in range(G):
                    iloc = G * g + i
                    l3 = l3p.tile([3, H], BF16, tag="lhsT3", name="lhsT3")
                    nc.gpsimd.tensor_scalar(
                        out=l3, in0=w1cb3_b,
                        scalar1=posrT_f[:, ds(iloc, 1)], scalar2=-2.0,
                        op0=ALU.mult, op1=ALU.mult)
                    for h in range(2):
                        ps = pp.tile([P, N], F32, tag="s1", name="s1", bufs=3)
                        nc.tensor.matmul(ps, l3[:, ds(P * h, P)], posT_b,
                                         start=True, stop=True)
                        nc.vector.scalar_tensor_tensor(
                            out=h1stg[h][:, ds(i * N, N)], in0=ps,
                            scalar=A2loc_f[h][:, ds(iloc, 1)], in1=BTp_f[h],
                            op0=ALU.add, op1=ALU.add)
                for h in range(2):
                    nc.scalar.activation(h1stg[h], h1stg[h], ACTF.Silu)

                # stage 2: msg = silu(eW2@h1 + b2) straight from PSUM on ACT;
                # accum_out computes agg = sum_j msg for free
                for i in range(G):
                    iloc = G * g + i
                    for h in range(2):
                        ps = pp.tile([P, N], F32, tag="mm", name="mm")
                        nc.tensor.matmul(ps, w2T_b[0][:, ds(P * h, P)],
                                         h1stg[0][:, ds(i * N, N)],
                                         start=True, stop=False)
                        nc.tensor.matmul(ps, w2T_b[1][:, ds(P * h, P)],
                                         h1stg[1][:, ds(i * N, N)],
                                         start=False, stop=True)
                        nc.scalar.activation(
                            msgstg[h][:, ds(i * N, N)], ps, ACTF.Silu,
                            bias=eb2c[h],
                            accum_out=aggT_f[h][:, ds(iloc, 1)])

                # stage 3: p1_pre = pW1 @ msg over 512-wide windows
                # (receiver-agnostic; fewer matmuls, amortized weight loads)
                W3 = 512
                for w0 in range(0, G * N, W3):
                    wl = min(W3, G * N - w0)
                    for h in range(2):
                        ps = pp.tile([P, W3], F32, tag="p1", name="p1")
                        nc.tensor.matmul(ps[:, 0:wl],
                                         pw1T_b[0][:, ds(P * h, P)],
                                         msgstg[0][:, ds(w0, wl)],
                                         start=True, stop=False)
                        nc.tensor.matmul(ps[:, 0:wl],
                                         pw1T_b[1][:, ds(P * h, P)],
                                         msgstg[1][:, ds(w0, wl)],
                                         start=False, stop=True)
                        nc.vector.tensor_copy(p1stg[h][:, ds(w0, wl)],
                                              ps[:, 0:wl])
                if g == NGRP - 1:
                    hn = G * N // 2
                    for h in range(2):
                        nc.scalar.activation(p1stg[h][:, 0:hn],
                                             p1stg[h][:, 0:hn],
                                             ACTF.Silu, bias=pb1c[h])
                    for h in range(2):
                        nc.scalar.activation(p1stg[h][:, ds(hn, hn)],
                                             p1stg[h][:, ds(hn, hn)],
                                             ACTF.Silu, bias=pb1c[h])
                else:
                    for h in range(2):
                        nc.scalar.activation(p1stg[h], p1stg[h], ACTF.Silu,
                                             bias=pb1c[h])

                # stage 4: psum4 += M4_i.T @ p1_i  (pb2 handled on host)
                for i in range(G):
                    iloc = G * g + i
                    first = (iloc == 0)
                    last = (iloc == NB - 1)
                    nc.tensor.matmul(p4ps, M4_b[0][:, ds(4 * iloc, 4)],
                                     p1stg[0][:, ds(i * N, N)],
                                     start=first, stop=False,
                                     skip_group_check=True)
                    nc.tensor.matmul(p4ps, M4_b[1][:, ds(4 * iloc, 4)],
                                     p1stg[1][:, ds(i * N, N)],
                                     start=False, stop=last,
                                     skip_group_check=True)

            nw1T_r = to_t(nw1T_f, "nw1T", F32R)
            nw2T_r = to_t(nw2T_f, "nw2T", F32R)

            # ============ node MLP + residual ============
            # agg := agg - msg_self, fused with the f32r cast
            aggT_r = []
            for h in range(2):
                t = cp.tile([P, NB], F32R, tag=f"aggT{h}c", name=f"aggT{h}c")
                nc.vector.tensor_sub(t, aggT_f[h], msgself_f[h])
                aggT_r.append(t)
            nh_r = []
            for h in range(2):
                ps = pp.tile([P, NB], F32, tag="s1", name="s1", bufs=3)
                nc.tensor.matmul(ps, nw1T_r[0][:, ds(P * h, P)], ndrT_r[0],
                                 start=True, stop=False)
                nc.tensor.matmul(ps, nw1T_r[1][:, ds(P * h, P)], ndrT_r[1],
                                 start=False, stop=False)
                nc.tensor.matmul(ps, nw1T_r[2][:, ds(P * h, P)], aggT_r[0],
                                 start=False, stop=False)
                nc.tensor.matmul(ps, nw1T_r[3][:, ds(P * h, P)], aggT_r[1],
                                 start=False, stop=True)
                t = cp.tile([P, NB], F32R, tag=f"nh{h}", name=f"nh{h}")
                nc.scalar.activation(t, ps, ACTF.Silu, bias=nb1c[h])
                nh_r.append(t)
            for h in range(2):
                ps = pp.tile([P, NB], F32, tag="p1", name="p1")
                nc.tensor.matmul(ps, nw2T_r[0][:, ds(P * h, P)], nh_r[0],
                                 start=True, stop=False)
                nc.tensor.matmul(ps, nw2T_r[1][:, ds(P * h, P)], nh_r[1],
                                 start=False, stop=True)
                t = cp.tile([P, NB], F32, tag=f"noutT{h}", name=f"noutT{h}")
                nc.vector.scalar_tensor_tensor(
                    out=t, in0=ps, scalar=nb2c[h], in1=ndrT_f[h],
                    op0=ALU.add, op1=ALU.add)
                dma(out=noutT_e[ds(P * h, P), :], in_=t)

            # ============ pos products out ============
            pout_f = cp.tile([4, N], F32, tag="pout", name="pout")
            nc.vector.tensor_copy(pout_f, p4ps)
            dma(out=pout_e[:, :], in_=pout_f)

    _split_waits(nc)
    return nc


_NC_CACHE = None


def _get_nc():
    global _NC_CACHE
    if _NC_CACHE is None:
        _NC_CACHE = build()
    return _NC_CACHE


def _make_in_maps(inputs):
    f32 = np.float32
    nodes = np.ascontiguousarray(np.asarray(inputs["nodes"], f32))
    pos = np.ascontiguousarray(np.asarray(inputs["pos"], f32))
    eW1 = np.asarray(inputs["eW1"], f32)
    eW2 = np.asarray(inputs["eW2"], f32)
    pW1 = np.asarray(inputs["pW1"], f32)
    pW2 = np.asarray(inputs["pW2"], f32)
    nW1 = np.asarray(inputs["nW1"], f32)
    nW2 = np.asarray(inputs["nW2"], f32)

    def c(x):
        return np.ascontiguousarray(x.astype(f32))

    def col2(v):
        return np.asarray(v, f32).reshape(H, 1).reshape(2, P).T  # [128, 2]

    eb1 = col2(inputs["eb1"]); eb2 = col2(inputs["eb2"])
    pb1 = col2(inputs["pb1"])
    nb1 = col2(inputs["nb1"]); nb2 = col2(inputs["nb2"])

    in_maps = []
    for cix in range(NCORES):
        blk = slice(NB * cix, NB * (cix + 1))
        packA = np.empty((P, CA), f32)

        def put(nm, arr):
            a = np.asarray(arr, f32)
            packA[:, _PA[nm]:_PA[nm] + a.shape[1]] = a

        ndT = nodes.T  # [256, 384]
        put("ndT", np.concatenate([ndT[:P], ndT[P:]], axis=1))
        ndrT = nodes[blk].T
        put("ndrT", np.concatenate([ndrT[:P], ndrT[P:]], axis=1))

        def wsplit(wT, nkc):  # [K, 256] -> [128, nkc*256]
            return np.concatenate([wT[P * k:P * (k + 1)] for k in range(nkc)],
                                  axis=1)

        put("w1aT", wsplit(eW1[:, :H].T, 2))
        put("w1bT", wsplit(eW1[:, H:2 * H].T, 2))
        put("w2T", wsplit(eW2.T, 2))
        put("pw1T", wsplit(pW1.T, 2))
        put("nw1T", wsplit(nW1.T, 4))
        put("nw2T", wsplit(nW2.T, 2))
        put("eb1", eb1); put("eb2", eb2); put("pb1", pb1)
        put("nb1", nb1); put("nb2", nb2)

        packB = np.zeros((3, CB), f32)
        packB[:, 0:N] = pos.T
        packB[:, _PB["posrT"]:_PB["posrT"] + NB] = pos[blk].T
        packB[0, _PB["w1c"]:_PB["w1c"] + H] = eW1[:, 2 * H]
        packB[0, _PB["pw2row"]:_PB["pw2row"] + H] = pW2.reshape(H)
        packB[0, _PB["pr4"]:_PB["pr4"] + 4 * NB] = np.concatenate(
            [pos[blk], np.ones((NB, 1), f32)], axis=1).reshape(4 * NB)
        in_maps.append({"packA": c(packA), "packB": c(packB)})
    return in_maps


def kernel(**inputs):
    f32 = np.float32
    nodes = np.ascontiguousarray(np.asarray(inputs["nodes"], f32))
    pos = np.ascontiguousarray(np.asarray(inputs["pos"], f32))
    in_maps = _make_in_maps(inputs)

    res = run_bass_kernel_spmd(_get_nc(), in_maps, list(range(NCORES))).results

    new_nodes = np.concatenate(
        [res[cix]["nodes_outT"].T for cix in range(NCORES)], axis=0)

    upd = np.zeros((N, 3), np.float64)
    pos64 = pos.astype(np.float64)
    for cix in range(NCORES):
        p4 = res[cix]["pos_out"].astype(np.float64)
        upd += pos64 * p4[3][:, None] - p4[0:3].T
    # pb2 enters every scale entry; its pos contribution is linear and exact:
    # sum_{i != j} (p_j - p_i) * pb2 = pb2 * (N * p_j - sum_i p_i)
    pb2 = float(np.asarray(inputs["pb2"]).reshape(-1)[0])
    upd += pb2 * (N * pos64 - pos64.sum(axis=0, keepdims=True))
    new_pos = (pos64 + upd).astype(f32)
    return new_nodes, new_pos
